# revision 1
# baseline (speedup 1.0000x reference)
"""Edge-parallel GNN message-passing MLP on 8 TRN2 NeuronCores.

Computation (per edge e): out[e] = relu(concat(x[row[e]], edge_attr[e]) @ W1 + b1) @ W2 + b2

Sharding: edges split evenly across the 8 cores (edge-parallel); x and the MLP
weights are replicated. Per core, per 2048-edge tile:
  - dma_gather fetches the x row-pair x2[row>>1] (512 B) for each edge
    (row-pair indexing keeps the gather indices within int16 range)
  - a parity select keeps the correct 256 B half; edge_attr is DMAed into the
    other half of the same edge-major tile
  - PE transposes 128x128 blocks to feature-major, then a 2-layer MLP runs in
    fp32r (full-rate fp32) with relu+bias fused on the scalar engine
  - results stream back as one contiguous 1 MiB store per tile

Tile edge mapping is partition-minor (edge = block*128 + partition) to match
dma_gather's output layout.

Self-contained: shapes/sharding are hardcoded for the 50000-node / 800000-edge
/ 64-feature problem instance.
"""

from contextlib import ExitStack

import numpy as np

import concourse.bacc as bacc_mod
import concourse.bass as bass
import concourse.mybir as mybir
import concourse.tile as tile
from concourse.bass_utils import run_bass_kernel_spmd
from concourse.masks import make_identity

N_CORES = 8
N_NODES = 50000
N_EDGES = 800000
F_IN = 64
HIDDEN = 128
F_OUT = 128

E_REAL = N_EDGES // N_CORES  # 100000 edges per core
TILE_E = 2048                # edges per pipeline tile
NT = 49                      # tiles per core
EPC = NT * TILE_E            # 100352 padded edges per core
KPT = TILE_E // 128          # 16 128-edge blocks per tile
QUARTER = 4                  # 128-edge blocks per PSUM-stage quarter

F32 = mybir.dt.float32
F32R = mybir.dt.float32r
I16 = mybir.dt.int16
I8 = mybir.dt.int8

RELU = mybir.ActivationFunctionType.Relu
ADD = mybir.AluOpType.add


def build_program(nt: int = NT):
    epc = nt * TILE_E
    nc = bacc_mod.Bacc("TRN2")

    # x viewed as row pairs: x2[i] = concat(x[2i], x[2i+1])
    x2_d = nc.declare_dram_parameter("x2", [N_NODES // 2, 2 * F_IN], F32, isOutput=False)
    # gather indices (row>>1) in dma_gather's [16, n/16] wrap, tiled to 128 partitions
    hidx_d = nc.declare_dram_parameter("hidx", [nt * 128, TILE_E // 16], I16, isOutput=False)
    # row parity as f32 mask, [tile, partition, block] layout
    par_d = nc.declare_dram_parameter("par", [nt * 128, KPT], I8, isOutput=False)
    ea_d = nc.declare_dram_parameter("ea", [epc, F_IN], F32, isOutput=False)
    w1_d = nc.declare_dram_parameter("w1", [2 * F_IN, HIDDEN], F32, isOutput=False)
    w2p_d = nc.declare_dram_parameter("w2p", [HIDDEN, 2 * F_OUT], F32, isOutput=False)
    b1_d = nc.declare_dram_parameter("b1c", [HIDDEN, 1], F32, isOutput=False)
    b2_d = nc.declare_dram_parameter("b2", [F_OUT], F32, isOutput=False)
    out_d = nc.declare_dram_parameter("out", [epc, F_OUT], F32, isOutput=True)

    # edge e = t*TILE_E + c*128 + p  <->  (tile t, partition p, block c)
    hidx_r = hidx_d[:, :].rearrange("(t p) s -> t p s", p=128)
    par_r = par_d[:, :].rearrange("(t p) c -> t p c", p=128)
    ea_r = ea_d[:, :].rearrange("(t c p) f -> t p c f", c=KPT, p=128)
    out_r = out_d[:, :].rearrange("(t c p) f -> t p c f", c=KPT, p=128)

    with tile.TileContext(nc) as tc, ExitStack() as ctx:
        const = ctx.enter_context(tc.tile_pool(name="const", bufs=1))
        idx_p = ctx.enter_context(tc.tile_pool(name="idx", bufs=2))
        xg2_p = ctx.enter_context(tc.tile_pool(name="xg2", bufs=2))
        feats_p = ctx.enter_context(tc.tile_pool(name="feats", bufs=2))
        ftsb_p = ctx.enter_context(tc.tile_pool(name="ftsb", bufs=2))
        h1sb_p = ctx.enter_context(tc.tile_pool(name="h1sb", bufs=2))
        outsb_p = ctx.enter_context(tc.tile_pool(name="outsb", bufs=2))
        ftps_p = ctx.enter_context(tc.tile_pool(name="ftps", bufs=2, space="PSUM"))
        h1ps_p = ctx.enter_context(tc.tile_pool(name="h1ps", bufs=2, space="PSUM"))
        outps_p = ctx.enter_context(tc.tile_pool(name="outps", bufs=2, space="PSUM"))

        # ---- constants (loaded once) ----
        w1_raw = const.tile([128, HIDDEN], F32, tag="w1_raw")
        nc.sync.dma_start(out=w1_raw, in_=w1_d[:, :])
        w1_t = const.tile([128, HIDDEN], F32R, tag="w1")
        nc.vector.tensor_copy(out=w1_t, in_=w1_raw)
        w2p_raw = const.tile([128, 2 * F_OUT], F32, tag="w2p_raw")
        nc.sync.dma_start(out=w2p_raw, in_=w2p_d[:, :])
        w2p_t = const.tile([128, 2 * F_OUT], F32R, tag="w2p")
        nc.vector.tensor_copy(out=w2p_t, in_=w2p_raw)
        b1_t = const.tile([128, 1], F32, tag="b1")
        nc.sync.dma_start(out=b1_t, in_=b1_d[:, :])
        # b2 replicated: [128 partitions, 4 blocks, 128] all copies of b2
        b2f_t = const.tile([128, QUARTER, F_OUT], F32, tag="b2f")
        b2_ap = b2_d[:]
        b2_bcast = bass.AP(b2_ap.tensor, b2_ap.offset, [[0, 128], [0, QUARTER], [1, F_OUT]])
        nc.gpsimd.dma_start(out=b2f_t, in_=b2_bcast)
        ident = const.tile([128, 128], F32, tag="ident")
        make_identity(nc, ident)

        for t in range(nt):
            # ---- load gather indices + parity mask ----
            idx16 = idx_p.tile([128, TILE_E // 16], I16, tag="idx16")
            nc.sync.dma_start(out=idx16, in_=hidx_r[t])
            part = idx_p.tile([128, KPT, 1], I8, tag="par")
            nc.sync.dma_start(out=part[:, :, 0], in_=par_r[t])

            # ---- gather x row pairs ----
            xg2 = xg2_p.tile([128, KPT, 2 * F_IN], F32, tag="xg2")
            nc.gpsimd.dma_gather(
                xg2[:, :, :],
                x2_d[:, :],
                idx16[:, :],
                TILE_E,
                TILE_E,
                2 * F_IN,
                single_packet=False,
            )

            # ---- build edge-major feats tile: [x_selected | edge_attr] ----
            feats = feats_p.tile([128, KPT, 2 * F_IN], F32, tag="feats")
            nc.scalar.copy(out=feats[:, :, 0:F_IN], in_=xg2[:, :, 0:F_IN])
            nc.vector.copy_predicated(
                out=feats[:, :, 0:F_IN],
                mask=part.to_broadcast([128, KPT, F_IN]),
                data=xg2[:, :, F_IN : 2 * F_IN],
            )
            nc.sync.dma_start(out=feats[:, :, F_IN : 2 * F_IN], in_=ea_r[t])

            h1sb = h1sb_p.tile([128, KPT, HIDDEN], F32R, tag="h1sb")
            out_sb = outsb_p.tile([128, KPT, F_OUT], F32, tag="out_sb")

            for q in range(KPT // QUARTER):
                # ---- transpose 4x [128 edges, 128 feats] -> [128 feats, 512 edges] ----
                ftps = ftps_p.tile([128, QUARTER * 128], F32, tag="ftps", space="PSUM")
                for j in range(QUARTER):
                    nc.tensor.transpose(
                        out=ftps[:, j * 128 : (j + 1) * 128],
                        in_=feats[:, q * QUARTER + j, :],
                        identity=ident,
                    )
                ftsb = ftsb_p.tile([128, QUARTER * 128], F32R, tag="ftsb")
                nc.vector.tensor_copy(out=ftsb, in_=ftps)

                # ---- layer 1: h1T[H, 512] = W1.T @ featsT ----
                h1ps = h1ps_p.tile([128, QUARTER * 128], F32, tag="h1ps", space="PSUM")
                nc.tensor.matmul(
                    out=h1ps,
                    lhsT=w1_t,
                    rhs=ftsb,
                    start=True,
                    stop=True,
                )
                nc.scalar.activation(
                    out=h1sb[:, q * QUARTER : (q + 1) * QUARTER, :],
                    in_=h1ps.rearrange("h (a b) -> h a b", a=QUARTER),
                    func=RELU,
                    bias=b1_t,
                    scale=1.0,
                )

                # ---- layer 2: out[128 edges, 256] = h1T_k.T @ W2pad ----
                outps = outps_p.tile([128, QUARTER, 2 * F_OUT], F32, tag="outps", space="PSUM")
                for j in range(QUARTER):
                    nc.tensor.matmul(
                        out=outps[:, j, :],
                        lhsT=h1sb[:, q * QUARTER + j, :],
                        rhs=w2p_t,
                        start=True,
                        stop=True,
                    )
                nc.vector.tensor_tensor(
                    out=out_sb[:, q * QUARTER : (q + 1) * QUARTER, :],
                    in0=outps[:, :, 0:F_OUT],
                    in1=b2f_t,
                    op=ADD,
                )

            nc.sync.dma_start(out=out_r[t], in_=out_sb)

    nc.compile()
    return nc


_PROG = None


def _get_prog():
    global _PROG
    if _PROG is None:
        _PROG = build_program(NT)
    return _PROG


def _prepare_in_maps(x, edge_index, edge_attr, W1, b1, W2, b2):
    x = np.ascontiguousarray(np.asarray(x, dtype=np.float32))
    row = np.ascontiguousarray(np.asarray(edge_index, dtype=np.int64)[0])
    ea = np.asarray(edge_attr, dtype=np.float32)
    w1 = np.ascontiguousarray(np.asarray(W1, dtype=np.float32))
    w2p = np.zeros((HIDDEN, 2 * F_OUT), dtype=np.float32)
    w2p[:, :F_OUT] = np.asarray(W2, dtype=np.float32)
    b1c = np.ascontiguousarray(np.asarray(b1, dtype=np.float32).reshape(HIDDEN, 1))
    b2v = np.ascontiguousarray(np.asarray(b2, dtype=np.float32).reshape(F_OUT))
    x2 = x.reshape(N_NODES // 2, 2 * F_IN)

    in_maps = []
    for c in range(N_CORES):
        sl = slice(c * E_REAL, (c + 1) * E_REAL)
        row_pad = np.zeros((EPC,), dtype=np.int64)
        row_pad[:E_REAL] = row[sl]
        ea_pad = np.zeros((EPC, F_IN), dtype=np.float32)
        ea_pad[:E_REAL] = ea[sl]
        # dma_gather index wrap: sequence pos i = s*16 + p16 read from idxs[p16, s];
        # within a tile, dest position i = c*128 + p  (partition-minor edge order)
        hr = (row_pad >> 1).astype(np.int16)
        hidx = np.ascontiguousarray(
            np.tile(hr.reshape(NT, TILE_E // 16, 16).transpose(0, 2, 1), (1, 8, 1))
        ).reshape(NT * 128, TILE_E // 16)
        par = (row_pad & 1).astype(np.int8)
        par_r = np.ascontiguousarray(
            par.reshape(NT, KPT, 128).transpose(0, 2, 1)
        ).reshape(NT * 128, KPT)
        in_maps.append(
            {
                "x2": x2,
                "hidx": hidx,
                "par": par_r,
                "ea": ea_pad,
                "w1": w1,
                "w2p": w2p,
                "b1c": b1c,
                "b2": b2v,
            }
        )
    return in_maps


def run_spmd(inputs: dict, trace: bool = False, **spmd_kwargs):
    """Run the kernel on all 8 cores. Returns (output, BassKernelResults)."""
    in_maps = _prepare_in_maps(
        inputs["x"], inputs["edge_index"], inputs["edge_attr"],
        inputs["W1"], inputs["b1"], inputs["W2"], inputs["b2"],
    )
    nc = _get_prog()
    bres = run_bass_kernel_spmd(
        nc, in_maps, list(range(N_CORES)), trace=trace, **spmd_kwargs
    )
    res = bres.results
    # undo the partition-minor edge order: output row e is already in natural
    # order (out_d is indexed by e directly), so just trim the padding
    out = np.concatenate([res[c]["out"][:E_REAL] for c in range(N_CORES)], axis=0)
    return np.ascontiguousarray(out, dtype=np.float32), bres


def kernel(x, edge_index, edge_attr, u, batch, W1, b1, W2, b2):
    out, _ = run_spmd(
        {
            "x": x, "edge_index": edge_index, "edge_attr": edge_attr,
            "W1": W1, "b1": b1, "W2": W2, "b2": b2,
        }
    )
    return out



# revision 7
# speedup vs baseline: 2.9548x; 2.9548x over previous
"""Edge-parallel GNN message-passing MLP on 8 TRN2 NeuronCores.

Computation (per edge e): out[e] = relu(concat(x[row[e]], edge_attr[e]) @ W1 + b1) @ W2 + b2

Sharding: edges are SORTED BY ROW on the host, then split evenly across the 8
cores (edge-parallel). Sorting keeps each core's rows in a contiguous band of
~6250 nodes (gather indices fit int16 against the band base) and makes
adjacent edges usually share a row.

Host-side stream construction (per core):
  - edges are packed into PAIRS that share a row (~3% dummy duplicate edges
    inserted at odd-length run boundaries), so one 256 B gather token serves
    two edges: halves gather descriptors and gather HBM traffic
  - x rows are zero-padded to 256 B fp16 tokens [64 feats | 64 zeros]
  - edge_attr is pre-transposed to [64, E] fp16 in stream order
  - the within-tile edge order is chosen so the output DMA writes 4 KiB
    contiguous DRAM per partition (8 consecutive rows), 8x fewer descriptors

Device per 2048-edge tile:
  - dma_gather (non-transpose, 256 B tokens, 1024 pair indices) on SWDGE
    queue t%4 — 4 Q7 core-pairs generate descriptors concurrently
  - PE "transpose" of each 128-pair block against a [128, 256] duplication
    matrix D (D[i,2i]=D[i,2i+1]=1) yields feature-major fp16 x features with
    each pair column already duplicated per edge
  - DVE copies them into the feats tile; edge_attr DMAs into partitions
    64:127; L1 fp16 matmul + fused relu+b1 on ScalarE; L2 per-block matmuls
    (FWL fp16 weight loads); DVE adds b2 -> fp32; 4 KiB-per-partition output
    DMA; host scatters rows back to original edge order.
"""

from contextlib import ExitStack

import numpy as np

import concourse.bacc as bacc_mod
import concourse.bass as bass
import concourse.mybir as mybir
import concourse.tile as tile
from concourse.bass_utils import run_bass_kernel_spmd

N_CORES = 8
N_NODES = 50000
N_EDGES = 800000
F_IN = 64
HIDDEN = 128
F_OUT = 128

E_REAL = N_EDGES // N_CORES  # 100000 edges per core
TILE_E = 2048                # edges per pipeline tile
PAIRS_T = TILE_E // 2        # 1024 gather tokens per tile
NT = 51                      # tiles per core (room for ~3% pairing dummies)
EPC = NT * TILE_E            # padded edge-stream length per core
HALF = 1024                  # edges per PSUM-stage half-tile
BAND = 8192                  # max x rows referenced per core (sorted band)

F32 = mybir.dt.float32
F16 = mybir.dt.float16
I16 = mybir.dt.int16

RELU = mybir.ActivationFunctionType.Relu
ADD = mybir.AluOpType.add


def build_program(nt: int = NT):
    epc = nt * TILE_E
    nc = bacc_mod.Bacc("TRN2", num_swdge_queues=4)

    xtok_d = nc.declare_dram_parameter("xtok", [BAND, 2 * F_IN], F16, isOutput=False)
    # pair indices (row - band_base), [16, n/16]-wrapped per tile, replicated
    # x8 across partitions so every SWDGE queue's Q7 pair sees its copy
    idx_d = nc.declare_dram_parameter("idx", [128, nt * (PAIRS_T // 16)], I16, isOutput=False)
    eat_d = nc.declare_dram_parameter("eat", [F_IN, epc], F16, isOutput=False)
    ident_d = nc.declare_dram_parameter("ident", [128, 128], F16, isOutput=False)
    w1_d = nc.declare_dram_parameter("w1", [2 * F_IN, HIDDEN], F16, isOutput=False)
    w2_d = nc.declare_dram_parameter("w2", [HIDDEN, F_OUT], F16, isOutput=False)
    b1_d = nc.declare_dram_parameter("b1c", [HIDDEN, 1], F32, isOutput=False)
    b2_d = nc.declare_dram_parameter("b2r", [128, F_OUT], F32, isOutput=False)
    out_d = nc.declare_dram_parameter("out", [epc, F_OUT], F32, isOutput=True)

    # half-tile n: DRAM row n*HALF + p*8 + j holds stream edge n*HALF + j*128 + p;
    # the DMA writes [128 parts, 8 rows x 512 B] as 4 KiB contiguous per partition
    out_r = out_d[:, :].rearrange("(n p j) f -> n p (j f)", p=128, j=HALF // 128)

    with tile.TileContext(nc) as tc, ExitStack() as ctx:
        const = ctx.enter_context(tc.tile_pool(name="const", bufs=1))
        xg_p = ctx.enter_context(tc.tile_pool(name="xg", bufs=6))
        feats_p = ctx.enter_context(tc.tile_pool(name="feats", bufs=4))
        h1sb_p = ctx.enter_context(tc.tile_pool(name="h1sb", bufs=3))
        outsb_p = ctx.enter_context(tc.tile_pool(name="outsb", bufs=3))
        xpt_p = ctx.enter_context(tc.tile_pool(name="xpt", bufs=2, space="PSUM"))
        h1ps_p = ctx.enter_context(tc.tile_pool(name="h1ps", bufs=2, space="PSUM"))
        outps_p = ctx.enter_context(tc.tile_pool(name="outps", bufs=2, space="PSUM"))

        # ---- constants (loaded once) ----
        w1_t = const.tile([128, HIDDEN], F16, tag="w1")
        nc.sync.dma_start(out=w1_t, in_=w1_d[:, :])
        w2_t = const.tile([128, F_OUT], F16, tag="w2")
        nc.sync.dma_start(out=w2_t, in_=w2_d[:, :])
        ident_t = const.tile([128, 128], F16, tag="ident")
        nc.sync.dma_start(out=ident_t, in_=ident_d[:, :])
        b1_t = const.tile([128, 1], F32, tag="b1")
        nc.sync.dma_start(out=b1_t, in_=b1_d[:, :])
        b2f_t = const.tile([128, 1, F_OUT], F32, tag="b2f")
        nc.sync.dma_start(out=b2f_t[:, 0, :], in_=b2_d[:, :])
        idx_t = const.tile([128, nt * (PAIRS_T // 16)], I16, tag="idx")
        nc.sync.dma_start(out=idx_t, in_=idx_d[:, :])

        S = PAIRS_T // 16
        for t in range(nt):
            # ---- gather pair tokens, pair-major [pair%128, pair//128, elem] ----
            xg = xg_p.tile([128, PAIRS_T // 128, 2 * F_IN], F16, tag="xg")
            nc.gpsimd.dma_gather(
                xg[:, :, :],
                xtok_d[:, :],
                idx_t[:, t * S : (t + 1) * S],
                PAIRS_T,
                PAIRS_T,
                2 * F_IN,
                transpose=False,
                single_packet=False,
                queue_num=t % 4,
            )

            # ---- feature-major via PE transposes: xpT[f, pair] ----
            xpt = xpt_p.tile([F_IN, PAIRS_T // 128, 128], F16, tag="xpt", space="PSUM")
            for b in range(PAIRS_T // 128):
                nc.tensor.transpose(
                    out=xpt[:, b, :],
                    in_=xg[:, b, 0:F_IN],
                    identity=ident_t,
                )
            # copy pair columns duplicated per edge (stride-0 dup dim), split
            # across DVE and ScalarE: feats col 256*b + 2*k + d = pair 128*b + k
            feats = feats_p.tile([128, TILE_E], F16, tag="feats")
            nb2 = PAIRS_T // 256  # blocks per engine half
            xap = xpt[:, :, :]
            dup_lo = bass.AP(xap.tensor, xap.offset, [xap.ap[0], [xap.ap[1][0], nb2], xap.ap[2], [0, 2]])
            hi = xpt[:, nb2:, :]
            dup_hi = bass.AP(hi.tensor, hi.offset, [hi.ap[0], [hi.ap[1][0], nb2], hi.ap[2], [0, 2]])
            nc.vector.tensor_copy(
                out=feats[0:F_IN, 0 : TILE_E // 2].rearrange(
                    "f (b e d) -> f b e d", b=nb2, e=128, d=2
                ),
                in_=dup_lo,
            )
            nc.scalar.copy(
                out=feats[0:F_IN, TILE_E // 2 : TILE_E].rearrange(
                    "f (b e d) -> f b e d", b=nb2, e=128, d=2
                ),
                in_=dup_hi,
            )
            nc.sync.dma_start(
                out=feats[F_IN : 2 * F_IN, :],
                in_=eat_d[:, t * TILE_E : (t + 1) * TILE_E],
            )

            for h in range(TILE_E // HALF):
                fh = feats[:, h * HALF : (h + 1) * HALF]

                # ---- layer 1: h1T[H, 1024] = W1.T @ feats ----
                h1ps = h1ps_p.tile([128, HALF], F32, tag="h1ps", space="PSUM")
                for q in range(HALF // 512):
                    nc.tensor.matmul(
                        out=h1ps[:, q * 512 : (q + 1) * 512],
                        lhsT=w1_t,
                        rhs=fh[:, q * 512 : (q + 1) * 512],
                        start=True,
                        stop=True,
                    )
                h1sb = h1sb_p.tile([128, HALF], F16, tag="h1sb")
                nc.scalar.activation(
                    out=h1sb, in_=h1ps, func=RELU, bias=b1_t, scale=1.0
                )

                # ---- layer 2 + bias: edge-major out blocks ----
                outsb = outsb_p.tile([128, HALF // 128, F_OUT], F32, tag="outsb")
                for half2 in range(2):
                    outps = outps_p.tile([128, 512], F32, tag="outps", space="PSUM")
                    for j in range(4):
                        jj = half2 * 4 + j
                        nc.tensor.matmul(
                            out=outps[:, j * 128 : (j + 1) * 128],
                            lhsT=h1sb[:, jj * 128 : (jj + 1) * 128],
                            rhs=w2_t,
                            start=True,
                            stop=True,
                        )
                    nc.vector.tensor_tensor(
                        out=outsb[:, half2 * 4 : (half2 + 1) * 4, :],
                        in0=outps.rearrange("e (c f) -> e c f", c=4),
                        in1=b2f_t.to_broadcast([128, 4, F_OUT]),
                        op=ADD,
                    )
                nc.sync.dma_start(
                    out=out_r[t * (TILE_E // HALF) + h],
                    in_=outsb.rearrange("p j f -> p (j f)"),
                )

    nc.compile()
    return nc


_PROG = None


def _get_prog():
    global _PROG
    if _PROG is None:
        _PROG = build_program(NT)
    return _PROG


def _pair_stream(rows_c):
    """Pack sorted rows into same-row pairs, duplicating the last edge of
    odd-length runs. Returns local edge indices, one per stream slot."""
    n = len(rows_c)
    change = np.flatnonzero(np.diff(rows_c)) + 1
    run_starts = np.concatenate([[0], change])
    run_lens = np.diff(np.concatenate([run_starts, [n]]))
    npairs = (run_lens + 1) // 2
    total_pairs = int(npairs.sum())
    pair_run = np.repeat(np.arange(len(run_lens)), npairs)
    first_pair = np.cumsum(npairs) - npairs
    pair_off = np.arange(total_pairs) - first_pair[pair_run]
    e0 = run_starts[pair_run] + 2 * pair_off
    e1 = np.minimum(e0 + 1, run_starts[pair_run] + run_lens[pair_run] - 1)
    stream = np.empty(2 * total_pairs, dtype=np.int64)
    stream[0::2] = e0
    stream[1::2] = e1
    return stream


def _prepare_in_maps(x, edge_index, edge_attr, W1, b1, W2, b2):
    row = np.ascontiguousarray(np.asarray(edge_index)[0]).astype(np.int64)
    order = np.argsort(row, kind="stable")
    row_s = row[order]
    ea_s = np.asarray(edge_attr, dtype=np.float32)[order]
    x16 = np.asarray(x, dtype=np.float32).astype(np.float16)
    w1_16 = np.ascontiguousarray(np.asarray(W1, dtype=np.float32).astype(np.float16))
    w2_16 = np.ascontiguousarray(np.asarray(W2, dtype=np.float32).astype(np.float16))
    b1c = np.ascontiguousarray(np.asarray(b1, dtype=np.float32).reshape(HIDDEN, 1))
    b2v = np.ascontiguousarray(
        np.broadcast_to(np.asarray(b2, dtype=np.float32).reshape(1, F_OUT), (128, F_OUT))
    )

    ident = np.eye(128, dtype=np.float16)

    in_maps = []
    streams = []
    for c in range(N_CORES):
        sl = slice(c * E_REAL, (c + 1) * E_REAL)
        rows_c = row_s[sl]
        r0 = int(rows_c[0])
        band_n = int(rows_c[-1]) - r0 + 1
        assert band_n <= BAND, (c, band_n)

        stream = _pair_stream(rows_c)
        assert len(stream) <= EPC, (c, len(stream))
        stream_pad = np.zeros(EPC, dtype=np.int64)
        stream_pad[: len(stream)] = stream
        streams.append((stream, len(stream)))

        pair_rows = (rows_c[stream_pad[0::2]] - r0).astype(np.int16)  # [EPC//2]
        idx_t = np.ascontiguousarray(
            np.tile(
                pair_rows.reshape(NT, PAIRS_T // 16, 16).transpose(0, 2, 1), (1, 8, 1)
            ).transpose(1, 0, 2)
        ).reshape(128, NT * (PAIRS_T // 16))

        xb = np.zeros((BAND, 2 * F_IN), dtype=np.float16)
        nb = min(BAND, N_NODES - r0)
        xb[:nb, :F_IN] = x16[r0 : r0 + nb]

        eat = np.ascontiguousarray(ea_s[sl][stream_pad].astype(np.float16).T)

        in_maps.append(
            {
                "xtok": xb,
                "idx": idx_t,
                "eat": eat,
                "ident": ident,
                "w1": w1_16,
                "w2": w2_16,
                "b1c": b1c,
                "b2r": b2v,
            }
        )
    return in_maps, order, streams


def run_spmd(inputs: dict, trace: bool = False, **spmd_kwargs):
    """Run the kernel on all 8 cores. Returns (output, BassKernelResults)."""
    in_maps, order, streams = _prepare_in_maps(
        inputs["x"], inputs["edge_index"], inputs["edge_attr"],
        inputs["W1"], inputs["b1"], inputs["W2"], inputs["b2"],
    )
    nc = _get_prog()
    bres = run_bass_kernel_spmd(
        nc, in_maps, list(range(N_CORES)), trace=trace, **spmd_kwargs
    )
    res = bres.results

    # stream position q (in half-tile n) -> DRAM row n*HALF + (q%128)*8 + (q%HALF)//128
    q = np.arange(EPC)
    dperm = (q // HALF) * HALF + (q % 128) * 8 + (q % HALF) // 128
    out = np.empty((N_EDGES, F_OUT), dtype=np.float32)
    for c in range(N_CORES):
        stream, slen = streams[c]
        core_out = res[c]["out"]  # [EPC, 128] in DRAM order
        sl_ids = order[c * E_REAL : (c + 1) * E_REAL]
        out[sl_ids[stream]] = core_out[dperm[:slen]]
    return out, bres


def kernel(x, edge_index, edge_attr, u, batch, W1, b1, W2, b2):
    out, _ = run_spmd(
        {
            "x": x, "edge_index": edge_index, "edge_attr": edge_attr,
            "W1": W1, "b1": b1, "W2": W2, "b2": b2,
        }
    )
    return out


# revision 9
# speedup vs baseline: 3.2630x; 1.1043x over previous
"""Edge-parallel GNN message-passing MLP on 8 TRN2 NeuronCores.

Computation (per edge e): out[e] = relu(concat(x[row[e]], edge_attr[e]) @ W1 + b1) @ W2 + b2

Sharding: edges are SORTED BY ROW on the host, then split evenly across the 8
cores (edge-parallel). Sorting keeps each core's rows in a contiguous band of
~6250 nodes (gather indices fit int16 against the band base) and makes
adjacent edges usually share a row.

Host-side stream construction (per core):
  - edges are packed into PAIRS that share a row (~3% dummy duplicate edges
    inserted at odd-length run boundaries), so one 256 B gather token serves
    two edges: halves gather descriptors and gather HBM traffic
  - x rows are zero-padded to 256 B fp16 tokens [64 feats | 64 zeros]
  - edge_attr is pre-transposed to [64, E] fp16 in stream order
  - the within-tile edge order is chosen so the output DMA writes 4 KiB
    contiguous DRAM per partition (8 consecutive rows), 8x fewer descriptors

Device per 2048-edge tile:
  - dma_gather (non-transpose, 256 B tokens, 1024 pair indices) on SWDGE
    queue t%4 — 4 Q7 core-pairs generate descriptors concurrently
  - PE "transpose" of each 128-pair block against a [128, 256] duplication
    matrix D (D[i,2i]=D[i,2i+1]=1) yields feature-major fp16 x features with
    each pair column already duplicated per edge
  - DVE copies them into the feats tile; edge_attr DMAs into partitions
    64:127; L1 fp16 matmul + fused relu+b1 on ScalarE; L2 per-block matmuls
    (FWL fp16 weight loads); DVE adds b2 -> fp32; 4 KiB-per-partition output
    DMA; host scatters rows back to original edge order.
"""

from contextlib import ExitStack

import numpy as np

import concourse.bacc as bacc_mod
import concourse.bass as bass
import concourse.mybir as mybir
import concourse.tile as tile
from concourse.bass_utils import run_bass_kernel_spmd

N_CORES = 8
N_NODES = 50000
N_EDGES = 800000
F_IN = 64
HIDDEN = 128
F_OUT = 128

E_REAL = N_EDGES // N_CORES  # 100000 edges per core
TILE_E = 2048                # edges per pipeline tile
PAIRS_T = TILE_E // 2        # 1024 gather tokens per tile
NT = 51                      # tiles per core (room for ~3% pairing dummies)
EPC = NT * TILE_E            # padded edge-stream length per core
HALF = 1024                  # edges per PSUM-stage half-tile
BAND = 8192                  # max x rows referenced per core (sorted band)

F32 = mybir.dt.float32
F16 = mybir.dt.float16
I16 = mybir.dt.int16

RELU = mybir.ActivationFunctionType.Relu
ADD = mybir.AluOpType.add


def build_program(nt: int = NT):
    epc = nt * TILE_E
    nc = bacc_mod.Bacc("TRN2", num_swdge_queues=4)

    xtok_d = nc.declare_dram_parameter("xtok", [BAND, 2 * F_IN], F16, isOutput=False)
    # pair indices (row - band_base), [16, n/16]-wrapped per tile, replicated
    # x8 across partitions so every SWDGE queue's Q7 pair sees its copy
    idx_d = nc.declare_dram_parameter("idx", [128, nt * (PAIRS_T // 16)], I16, isOutput=False)
    eat_d = nc.declare_dram_parameter("eat", [F_IN, epc], F16, isOutput=False)
    ident_d = nc.declare_dram_parameter("ident", [128, 128], F16, isOutput=False)
    w1_d = nc.declare_dram_parameter("w1", [2 * F_IN, HIDDEN], F16, isOutput=False)
    w2_d = nc.declare_dram_parameter("w2", [HIDDEN, F_OUT], F16, isOutput=False)
    b1_d = nc.declare_dram_parameter("b1c", [HIDDEN, 1], F32, isOutput=False)
    b2_d = nc.declare_dram_parameter("b2c", [F_OUT, 1], F32, isOutput=False)
    # feature-major output: column q = stream edge q; host transposes back
    out_d = nc.declare_dram_parameter("out", [F_OUT, epc], F32, isOutput=True)

    with tile.TileContext(nc) as tc, ExitStack() as ctx:
        const = ctx.enter_context(tc.tile_pool(name="const", bufs=1))
        xg_p = ctx.enter_context(tc.tile_pool(name="xg", bufs=6))
        feats_p = ctx.enter_context(tc.tile_pool(name="feats", bufs=4))
        h1sb_p = ctx.enter_context(tc.tile_pool(name="h1sb", bufs=3))
        outsb_p = ctx.enter_context(tc.tile_pool(name="outsb", bufs=3))
        xpt_p = ctx.enter_context(tc.tile_pool(name="xpt", bufs=2, space="PSUM"))
        h1ps_p = ctx.enter_context(tc.tile_pool(name="h1ps", bufs=2, space="PSUM"))
        outps_p = ctx.enter_context(tc.tile_pool(name="outps", bufs=2, space="PSUM"))

        # ---- constants (loaded once) ----
        w1_t = const.tile([128, HIDDEN], F16, tag="w1")
        nc.sync.dma_start(out=w1_t, in_=w1_d[:, :])
        w2_t = const.tile([128, F_OUT], F16, tag="w2")
        nc.sync.dma_start(out=w2_t, in_=w2_d[:, :])
        ident_t = const.tile([128, 128], F16, tag="ident")
        nc.sync.dma_start(out=ident_t, in_=ident_d[:, :])
        b1_t = const.tile([128, 1], F32, tag="b1")
        nc.sync.dma_start(out=b1_t, in_=b1_d[:, :])
        b2_t = const.tile([128, 1], F32, tag="b2")
        nc.sync.dma_start(out=b2_t, in_=b2_d[:, :])
        idx_t = const.tile([128, nt * (PAIRS_T // 16)], I16, tag="idx")
        nc.sync.dma_start(out=idx_t, in_=idx_d[:, :])

        S = PAIRS_T // 16
        for t in range(nt):
            # ---- gather pair tokens, pair-major [pair%128, pair//128, elem] ----
            xg = xg_p.tile([128, PAIRS_T // 128, 2 * F_IN], F16, tag="xg")
            nc.gpsimd.dma_gather(
                xg[:, :, :],
                xtok_d[:, :],
                idx_t[:, t * S : (t + 1) * S],
                PAIRS_T,
                PAIRS_T,
                2 * F_IN,
                transpose=False,
                single_packet=False,
                queue_num=t % 4,
            )

            # ---- feature-major via PE transposes: xpT[f, pair] ----
            xpt = xpt_p.tile([F_IN, PAIRS_T // 128, 128], F16, tag="xpt", space="PSUM")
            for b in range(PAIRS_T // 128):
                nc.tensor.transpose(
                    out=xpt[:, b, :],
                    in_=xg[:, b, 0:F_IN],
                    identity=ident_t,
                )
            # copy pair columns duplicated per edge (stride-0 dup dim), split
            # across DVE and ScalarE: feats col 256*b + 2*k + d = pair 128*b + k
            feats = feats_p.tile([128, TILE_E], F16, tag="feats")
            nb2 = PAIRS_T // 256  # blocks per engine half
            xap = xpt[:, :, :]
            dup_lo = bass.AP(xap.tensor, xap.offset, [xap.ap[0], [xap.ap[1][0], nb2], xap.ap[2], [0, 2]])
            hi = xpt[:, nb2:, :]
            dup_hi = bass.AP(hi.tensor, hi.offset, [hi.ap[0], [hi.ap[1][0], nb2], hi.ap[2], [0, 2]])
            nc.vector.tensor_copy(
                out=feats[0:F_IN, 0 : TILE_E // 2].rearrange(
                    "f (b e d) -> f b e d", b=nb2, e=128, d=2
                ),
                in_=dup_lo,
            )
            nc.scalar.copy(
                out=feats[0:F_IN, TILE_E // 2 : TILE_E].rearrange(
                    "f (b e d) -> f b e d", b=nb2, e=128, d=2
                ),
                in_=dup_hi,
            )
            nc.sync.dma_start(
                out=feats[F_IN : 2 * F_IN, :],
                in_=eat_d[:, t * TILE_E : (t + 1) * TILE_E],
            )

            outsb = outsb_p.tile([128, TILE_E], F32, tag="outsb")
            for h in range(TILE_E // HALF):
                fh = feats[:, h * HALF : (h + 1) * HALF]

                # ---- layer 1: h1T[H, 1024] = W1.T @ feats ----
                h1ps = h1ps_p.tile([128, HALF], F32, tag="h1ps", space="PSUM")
                for q in range(HALF // 512):
                    nc.tensor.matmul(
                        out=h1ps[:, q * 512 : (q + 1) * 512],
                        lhsT=w1_t,
                        rhs=fh[:, q * 512 : (q + 1) * 512],
                        start=True,
                        stop=True,
                    )
                h1sb = h1sb_p.tile([128, HALF], F16, tag="h1sb")
                nc.scalar.activation(
                    out=h1sb, in_=h1ps, func=RELU, bias=b1_t, scale=1.0
                )

                # ---- layer 2 + bias, feature-major (W2 stationary) ----
                for half2 in range(2):
                    outps = outps_p.tile([128, 512], F32, tag="outps", space="PSUM")
                    nc.tensor.matmul(
                        out=outps,
                        lhsT=w2_t,
                        rhs=h1sb[:, half2 * 512 : (half2 + 1) * 512],
                        start=True,
                        stop=True,
                    )
                    nc.vector.tensor_tensor(
                        out=outsb[:, h * HALF + half2 * 512 : h * HALF + (half2 + 1) * 512],
                        in0=outps,
                        in1=b2_t.to_broadcast([128, 512]),
                        op=ADD,
                    )
            nc.sync.dma_start(
                out=out_d[:, t * TILE_E : (t + 1) * TILE_E],
                in_=outsb,
            )

    nc.compile()
    return nc


_PROG = None


def _get_prog():
    global _PROG
    if _PROG is None:
        _PROG = build_program(NT)
    return _PROG


def _pair_stream(rows_c):
    """Pack sorted rows into same-row pairs, duplicating the last edge of
    odd-length runs. Returns local edge indices, one per stream slot."""
    n = len(rows_c)
    change = np.flatnonzero(np.diff(rows_c)) + 1
    run_starts = np.concatenate([[0], change])
    run_lens = np.diff(np.concatenate([run_starts, [n]]))
    npairs = (run_lens + 1) // 2
    total_pairs = int(npairs.sum())
    pair_run = np.repeat(np.arange(len(run_lens)), npairs)
    first_pair = np.cumsum(npairs) - npairs
    pair_off = np.arange(total_pairs) - first_pair[pair_run]
    e0 = run_starts[pair_run] + 2 * pair_off
    e1 = np.minimum(e0 + 1, run_starts[pair_run] + run_lens[pair_run] - 1)
    stream = np.empty(2 * total_pairs, dtype=np.int64)
    stream[0::2] = e0
    stream[1::2] = e1
    return stream


def _prepare_in_maps(x, edge_index, edge_attr, W1, b1, W2, b2):
    row = np.ascontiguousarray(np.asarray(edge_index)[0]).astype(np.int64)
    order = np.argsort(row, kind="stable")
    row_s = row[order]
    ea_s = np.asarray(edge_attr, dtype=np.float32)[order]
    x16 = np.asarray(x, dtype=np.float32).astype(np.float16)
    w1_16 = np.ascontiguousarray(np.asarray(W1, dtype=np.float32).astype(np.float16))
    w2_16 = np.ascontiguousarray(np.asarray(W2, dtype=np.float32).astype(np.float16))
    b1c = np.ascontiguousarray(np.asarray(b1, dtype=np.float32).reshape(HIDDEN, 1))
    b2v = np.ascontiguousarray(np.asarray(b2, dtype=np.float32).reshape(F_OUT, 1))

    ident = np.eye(128, dtype=np.float16)

    in_maps = []
    streams = []
    for c in range(N_CORES):
        sl = slice(c * E_REAL, (c + 1) * E_REAL)
        rows_c = row_s[sl]
        r0 = int(rows_c[0])
        band_n = int(rows_c[-1]) - r0 + 1
        assert band_n <= BAND, (c, band_n)

        stream = _pair_stream(rows_c)
        assert len(stream) <= EPC, (c, len(stream))
        stream_pad = np.zeros(EPC, dtype=np.int64)
        stream_pad[: len(stream)] = stream
        streams.append((stream, len(stream)))

        pair_rows = (rows_c[stream_pad[0::2]] - r0).astype(np.int16)  # [EPC//2]
        idx_t = np.ascontiguousarray(
            np.tile(
                pair_rows.reshape(NT, PAIRS_T // 16, 16).transpose(0, 2, 1), (1, 8, 1)
            ).transpose(1, 0, 2)
        ).reshape(128, NT * (PAIRS_T // 16))

        xb = np.zeros((BAND, 2 * F_IN), dtype=np.float16)
        nb = min(BAND, N_NODES - r0)
        xb[:nb, :F_IN] = x16[r0 : r0 + nb]

        eat = np.ascontiguousarray(ea_s[sl][stream_pad].astype(np.float16).T)

        in_maps.append(
            {
                "xtok": xb,
                "idx": idx_t,
                "eat": eat,
                "ident": ident,
                "w1": w1_16,
                "w2": w2_16,
                "b1c": b1c,
                "b2c": b2v,
            }
        )
    return in_maps, order, streams


def run_spmd(inputs: dict, trace: bool = False, **spmd_kwargs):
    """Run the kernel on all 8 cores. Returns (output, BassKernelResults)."""
    in_maps, order, streams = _prepare_in_maps(
        inputs["x"], inputs["edge_index"], inputs["edge_attr"],
        inputs["W1"], inputs["b1"], inputs["W2"], inputs["b2"],
    )
    nc = _get_prog()
    bres = run_bass_kernel_spmd(
        nc, in_maps, list(range(N_CORES)), trace=trace, **spmd_kwargs
    )
    res = bres.results

    out = np.empty((N_EDGES, F_OUT), dtype=np.float32)
    for c in range(N_CORES):
        stream, slen = streams[c]
        core_out = res[c]["out"]  # [128, EPC] feature-major, col q = stream edge q
        sl_ids = order[c * E_REAL : (c + 1) * E_REAL]
        out[sl_ids[stream]] = core_out[:, :slen].T
    return out, bres


def kernel(x, edge_index, edge_attr, u, batch, W1, b1, W2, b2):
    out, _ = run_spmd(
        {
            "x": x, "edge_index": edge_index, "edge_attr": edge_attr,
            "W1": W1, "b1": b1, "W2": W2, "b2": b2,
        }
    )
    return out
